# revision 4
# baseline (speedup 1.0000x reference)
"""Trainium2 Bass kernel for local-window multihead attention.

Problem: B=8, L=1024, C=1024, H=16 heads, head_dim=64, window_size=128
(positions attend to |i-j| <= 64). qkv in-projection + banded softmax
attention + out-projection.

Sharding: data-parallel — one batch element per NeuronCore (8 cores).

Per-core dataflow (all matmuls bf16, fp32 PSUM accumulation):
  xT (host-transposed, bf16)  --QK-proj-->  qT/ktpad   [channel, seq] layout
                              --V-proj -->  vpad       [seq, channel] layout
  attention per (head, qtile): S = Q.K^T (banded, 256-wide window), exp on
  ScalarE, mask-mul + rowsum fused on VectorE (tensor_tensor_reduce),
  normalize, PE-transpose P -> P^T, O^T = V^T @ P^T, assemble oT [c, l]
  out-proj from oT -> y [l, c] fp32.
"""

import numpy as np
import ml_dtypes

import concourse.bass as bass
import concourse.mybir as mybir
import concourse.tile as tile
from concourse import bacc
from concourse.bass_utils import run_bass_kernel_spmd
from concourse.masks import make_identity

BF16 = mybir.dt.bfloat16
F32 = mybir.dt.float32

B, L, C, H, HD = 8, 1024, 1024, 16, 64
WIN = 128  # attention window (|i-j| <= WIN//2)
NCORES = 8
NQT = L // 128          # query tiles of 128 rows
NCT = C // 128          # channel tiles
SCALE = 1.0 / 8.0       # 1/sqrt(HD)

_CACHED = {}


def _build_nc():
    nc = bacc.Bacc(
        "TRN2", target_bir_lowering=False, debug=False, num_devices=NCORES
    )

    xT_d = nc.dram_tensor("xT", [C, L], BF16, kind="ExternalInput").ap()
    wqk_d = nc.dram_tensor("wqkT", [C, 3 * C], BF16, kind="ExternalInput").ap()
    wo_d = nc.dram_tensor("woT", [C, C], BF16, kind="ExternalInput").ap()
    bqk_d = nc.dram_tensor("bqk", [2 * C], F32, kind="ExternalInput").ap()
    bv_d = nc.dram_tensor("bv", [C], F32, kind="ExternalInput").ap()
    y_d = nc.dram_tensor("y", [L, C], F32, kind="ExternalOutput").ap()

    AF = mybir.ActivationFunctionType
    ALU = mybir.AluOpType

    with tile.TileContext(nc) as tc:
        with (
            tc.tile_pool(name="const", bufs=1) as const,
            tc.tile_pool(name="work", bufs=3) as work,
            tc.tile_pool(name="psum", bufs=2, space="PSUM") as psum,
        ):
            # ---- persistent SBUF tensors ----
            wqk_s = const.tile([128, NCT, 3 * C], BF16, tag="wqk")  # [c_p, ct, o]
            wo_s = const.tile([128, NCT, C], BF16, tag="wo")
            xT_s = const.tile([128, NCT, L], BF16, tag="xT")        # [c_p, ct, l]
            qT_s = const.tile([128, NQT, L], BF16, tag="qT")        # [q-chan, ot, l]
            ktp_s = const.tile([128, NQT, L + 128], BF16, tag="ktp")  # k padded +-64
            vp_s = const.tile([128, NQT + 1, C], BF16, tag="vp")    # v rows shifted +64
            oT_s = const.tile([128, NCT, L], BF16, tag="oT")        # [c_p, ct, l]
            bqk_s = const.tile([128, 2 * C // 128], F32, tag="bqk")
            bv_row = const.tile([1, C], F32, tag="bvrow")
            bvrep_s = const.tile([128, C], F32, tag="bvrep")
            ones1_s = const.tile([1, 128], F32, tag="ones1")
            masks_s = const.tile([128, 3, 256], BF16, tag="masks")
            id01_s = const.tile([128, 128], BF16, tag="id01")

            # ---- input DMAs ----
            nc.sync.dma_start(wqk_s[:], wqk_d.rearrange("(ct p) o -> p ct o", p=128))
            nc.sync.dma_start(wo_s[:], wo_d.rearrange("(ct p) o -> p ct o", p=128))
            nc.sync.dma_start(xT_s[:], xT_d.rearrange("(ct p) l -> p ct l", p=128))
            nc.sync.dma_start(bqk_s[:], bqk_d.rearrange("(ot p) -> p ot", p=128))
            nc.sync.dma_start(bv_row[:], bv_d.rearrange("(p c) -> p c", p=1))

            # ---- constants: identity, band masks, padded-region zeros ----
            make_identity(nc, id01_s[:])
            nc.gpsimd.memset(ones1_s[:], 1.0)

            # band mask m1: valid iff 0 <= jl - il <= 128 (window cols at
            # ktp offset qi*128 .. +256, jl = col - (qi*128), il = row)
            m0, m1, m2 = (masks_s[:, i, :] for i in range(3))
            nc.gpsimd.memset(m1, 1.0)
            nc.gpsimd.affine_select(  # keep where jl - il >= 0
                m1, m1, compare_op=ALU.is_ge, fill=0.0,
                base=0, pattern=[[1, 256]], channel_multiplier=-1,
            )
            nc.gpsimd.affine_select(  # keep where 128 - jl + il >= 0
                m1, m1, compare_op=ALU.is_ge, fill=0.0,
                base=128, pattern=[[-1, 256]], channel_multiplier=1,
            )
            # qi == 0: also need jl >= 64 (left zero-pad region invalid)
            nc.vector.tensor_copy(m0, m1)
            nc.gpsimd.affine_select(
                m0, m0, compare_op=ALU.is_ge, fill=0.0,
                base=-64, pattern=[[1, 256]], channel_multiplier=0,
            )
            # qi == NQT-1: also need jl <= 191 (right zero-pad invalid)
            nc.vector.tensor_copy(m2, m1)
            nc.gpsimd.affine_select(
                m2, m2, compare_op=ALU.is_ge, fill=0.0,
                base=191, pattern=[[-1, 256]], channel_multiplier=0,
            )

            # zero the +-64 padded edges of ktpad / vpad
            for ot in range(NQT):
                nc.gpsimd.memset(ktp_s[:, ot, 0:64], 0.0)
                nc.gpsimd.memset(ktp_s[:, ot, L + 64 : L + 128], 0.0)
            nc.gpsimd.memset(vp_s[0:64, 0, :], 0.0)
            nc.gpsimd.memset(vp_s[64:128, NQT, :], 0.0)

            # replicate v-bias across partitions: ones[128,1] @ bv[1,512]
            for nt in range(2):
                ps = psum.tile([128, 512], F32, tag="proj")
                nc.tensor.matmul(
                    ps[:], lhsT=ones1_s[:], rhs=bv_row[:, nt * 512 : (nt + 1) * 512],
                    start=True, stop=True,
                )
                nc.scalar.copy(bvrep_s[:, nt * 512 : (nt + 1) * 512], ps[:])

            # ---- phase 1: Q/K projection -> qT_s / ktp_s (transposed) ----
            for ot in range(2 * NQT):
                for lt in range(2):
                    ps = psum.tile([128, 512], F32, tag="proj")
                    for ct in range(NCT):
                        nc.tensor.matmul(
                            ps[:],
                            lhsT=wqk_s[:, ct, ot * 128 : (ot + 1) * 128],
                            rhs=xT_s[:, ct, lt * 512 : (lt + 1) * 512],
                            start=(ct == 0), stop=(ct == NCT - 1),
                        )
                    if ot < NQT:
                        dest = qT_s[:, ot, lt * 512 : (lt + 1) * 512]
                    else:
                        dest = ktp_s[:, ot - NQT, 64 + lt * 512 : 64 + (lt + 1) * 512]
                    nc.scalar.activation(
                        dest, ps[:], AF.Identity, bias=bqk_s[:, ot : ot + 1]
                    )

            # ---- phase 2: V projection -> vpad (seq-major, shifted +64) ----
            for lt in range(NQT):
                for nt in range(2):
                    ps = psum.tile([128, 512], F32, tag="proj")
                    for ct in range(NCT):
                        nc.tensor.matmul(
                            ps[:],
                            lhsT=xT_s[:, ct, lt * 128 : (lt + 1) * 128],
                            rhs=wqk_s[:, ct, 2 * C + nt * 512 : 2 * C + (nt + 1) * 512],
                            start=(ct == 0), stop=(ct == NCT - 1),
                        )
                    vtmp = work.tile([128, 512], BF16, tag="vtmp")
                    nc.vector.scalar_tensor_tensor(
                        out=vtmp[:], in0=ps[:], scalar=1.0,
                        in1=bvrep_s[:, nt * 512 : (nt + 1) * 512],
                        op0=ALU.mult, op1=ALU.add,
                    )
                    sl = slice(nt * 512, (nt + 1) * 512)
                    # rows lt*128+p shift to vpad row +64: split partition halves
                    nc.sync.dma_start(vp_s[64:128, lt, sl], vtmp[0:64, :])
                    nc.sync.dma_start(vp_s[0:64, lt + 1, sl], vtmp[64:128, :])

            # ---- phase 3: banded attention ----
            for hp in range(H // 2):          # head pairs share a 128-chan tile
                for qi in range(NQT):
                    mi = 0 if qi == 0 else (2 if qi == NQT - 1 else 1)
                    for hh in range(2):
                        hb = hh * 64
                        h = hp * 2 + hh
                        s_ps = psum.tile([128, 256], F32, tag="s")
                        nc.tensor.matmul(
                            s_ps[:],
                            lhsT=qT_s[hb : hb + 64, hp, qi * 128 : (qi + 1) * 128],
                            rhs=ktp_s[hb : hb + 64, hp, qi * 128 : qi * 128 + 256],
                            start=True, stop=True,
                        )
                        p_sb = work.tile([128, 256], BF16, tag="p")
                        nc.scalar.activation(p_sb[:], s_ps[:], AF.Exp, scale=SCALE)
                        # fused: pm = p * mask, rs = rowsum(pm)
                        pm = work.tile([128, 256], BF16, tag="pm")
                        rs = work.tile([128, 1], F32, tag="rs")
                        nc.vector.scalar_tensor_tensor(
                            out=pm[:], in0=p_sb[:], scalar=1.0,
                            in1=masks_s[:, mi, :],
                            op0=ALU.mult, op1=ALU.mult, accum_out=rs[:],
                        )
                        rc = work.tile([128, 1], F32, tag="rc")
                        nc.vector.reciprocal(rc[:], rs[:])
                        pn = work.tile([128, 256], BF16, tag="pn")
                        nc.vector.tensor_scalar_mul(pn[:], pm[:], rc[:])
                        pt1 = psum.tile([128, 128], BF16, tag="pt1", bufs=1)
                        pt2 = psum.tile([128, 128], BF16, tag="pt2", bufs=1)
                        nc.tensor.transpose(pt1[:], pn[:, 0:128], id01_s[:])
                        nc.tensor.transpose(pt2[:], pn[:, 128:256], id01_s[:])
                        pt_sb = work.tile([128, 256], BF16, tag="pt_sb")
                        nc.scalar.copy(pt_sb[:, 0:128], pt1[:])
                        nc.scalar.copy(pt_sb[:, 128:256], pt2[:])
                        ot_ps = psum.tile([128, 128], F32, tag="ot")
                        o_out = ot_ps[hb : hb + 64, :]
                        nc.tensor.matmul(
                            o_out,
                            lhsT=vp_s[:, qi, h * 64 : (h + 1) * 64],
                            rhs=pt_sb[:, 0:128], start=True, stop=False,
                        )
                        nc.tensor.matmul(
                            o_out,
                            lhsT=vp_s[:, qi + 1, h * 64 : (h + 1) * 64],
                            rhs=pt_sb[:, 128:256], start=False, stop=True,
                        )
                        nc.scalar.copy(
                            oT_s[hb : hb + 64, hp, qi * 128 : (qi + 1) * 128], o_out
                        )

            # ---- phase 4: out projection -> y ----
            for lt in range(NQT):
                for mt in range(2):
                    ps = psum.tile([128, 512], F32, tag="proj")
                    for ct in range(NCT):
                        nc.tensor.matmul(
                            ps[:],
                            lhsT=oT_s[:, ct, lt * 128 : (lt + 1) * 128],
                            rhs=wo_s[:, ct, mt * 512 : (mt + 1) * 512],
                            start=(ct == 0), stop=(ct == NCT - 1),
                        )
                    yb = work.tile([128, 512], F32, tag="yb")
                    nc.scalar.copy(yb[:], ps[:])
                    nc.sync.dma_start(
                        y_d[lt * 128 : (lt + 1) * 128, mt * 512 : (mt + 1) * 512],
                        yb[:],
                    )

    nc.compile()
    return nc


def _get_nc():
    if "nc" not in _CACHED:
        _CACHED["nc"] = _build_nc()
    return _CACHED["nc"]


def _prep_in_maps(x, in_proj_w, in_proj_b, out_w):
    bf = ml_dtypes.bfloat16
    wqkT = np.ascontiguousarray(in_proj_w.T).astype(bf)
    woT = np.ascontiguousarray(out_w.T).astype(bf)
    bqk = np.ascontiguousarray(in_proj_b[: 2 * C]).astype(np.float32)
    bv = np.ascontiguousarray(in_proj_b[2 * C :]).astype(np.float32)
    in_maps = []
    for b in range(B):
        xT = np.ascontiguousarray(x[b].T).astype(bf)
        in_maps.append(
            {"xT": xT, "wqkT": wqkT, "woT": woT, "bqk": bqk, "bv": bv}
        )
    return in_maps


def kernel(x, in_proj_w, in_proj_b, out_w, out_b, _trace=False):
    nc = _get_nc()
    in_maps = _prep_in_maps(x, in_proj_w, in_proj_b, out_w)
    res = run_bass_kernel_spmd(nc, in_maps, list(range(NCORES)), trace=_trace)
    _CACHED["last_result"] = res
    y = np.stack([res.results[i]["y"] for i in range(NCORES)], axis=0)
    return (y + out_b[None, None, :].astype(np.float32)).astype(np.float32)
